# revision 1
# baseline (speedup 1.0000x reference)
"""Performer (FAVOR+) attention on 8 trn2 NeuronCores.

Sharding: tensor-parallel over the 16 heads. Primary path uses jit+GSPMD
with X replicated via a single device_put and the QKV projections
column-sharded (2 heads / core), so every op partitions head-locally with
no collectives. Falls back to an equivalent jax.pmap implementation if
the GSPMD path fails.
"""
import numpy as np
import jax
import jax.numpy as jnp
from jax.sharding import Mesh, NamedSharding, PartitionSpec as P

B, S, D = 4, 4096, 1024
H = 16
HD = 64          # head dim
M = 256          # nb random features
N_CORES = 8
HPC = H // N_CORES          # heads per core = 2
COLS = HPC * HD             # projection columns per core = 128


def _feat(x, proj, is_query):
    ratio = M ** -0.5
    x = x * (HD ** -0.25)
    u = jnp.einsum('bhsd,md->bhsm', x, proj)
    diag = 0.5 * jnp.sum(x * x, axis=-1, keepdims=True)
    if is_query:
        stab = jnp.max(u, axis=-1, keepdims=True)
    else:
        stab = jnp.max(u, axis=(-1, -2), keepdims=True)
    return ratio * (jnp.exp(u - diag - stab) + 1e-4)


def _attn(Q, K, V, mask, proj):
    # Q,K,V: [B,h,S,HD] for any number of heads h
    scale = HD ** -0.25
    m4 = mask[:, None, :, None]
    Qs = Q * scale
    Ks = K * scale * m4
    Vs = V * m4
    q_prime = _feat(Qs, proj, True)
    k_prime = _feat(Ks, proj, False)
    kv = jnp.einsum('bhsm,bhsd->bhmd', k_prime, Vs)
    z = 1.0 / (jnp.einsum('bhsm,bhm->bhs', q_prime, jnp.sum(k_prime, axis=2)) + 1e-6)
    return jnp.einsum('bhsm,bhmd->bhsd', q_prime, kv) * z[..., None]


# ----- primary: jit + GSPMD, full-shape math, head-sharded via weight cols -----

def _compute_full(X, mask, Wq, bq, Wk, bk, Wv, bv, proj):
    def split(x):  # [B,S,D] -> [B,H,S,HD]
        return x.reshape(B, S, H, HD).transpose(0, 2, 1, 3)

    Q = split(X @ Wq + bq)
    K = split(X @ Wk + bk)
    V = split(X @ Wv + bv)
    out = _attn(Q, K, V, mask, proj)          # [B,H,S,HD]
    return out.transpose(0, 2, 1, 3).reshape(B, S, D)


_gspmd = None


def _run_gspmd(X, mask, Wq, bq, Wk, bk, Wv, bv, proj):
    global _gspmd
    devs = jax.devices()[:N_CORES]
    mesh = Mesh(np.array(devs), ('x',))
    rep = NamedSharding(mesh, P())
    col = NamedSharding(mesh, P(None, 'x'))
    vec = NamedSharding(mesh, P('x'))
    seq = NamedSharding(mesh, P(None, 'x', None))  # X sharded over S: 1 host copy
    outsh = NamedSharding(mesh, P(None, None, 'x'))
    if _gspmd is None:
        _gspmd = jax.jit(
            _compute_full,
            in_shardings=(seq, rep, col, vec, col, vec, col, vec, rep),
            out_shardings=outsh,
        )
    args = (
        jax.device_put(np.asarray(X, np.float32), seq),
        jax.device_put(np.asarray(mask, np.float32), rep),
        jax.device_put(np.asarray(Wq, np.float32), col),
        jax.device_put(np.asarray(bq, np.float32), vec),
        jax.device_put(np.asarray(Wk, np.float32), col),
        jax.device_put(np.asarray(bk, np.float32), vec),
        jax.device_put(np.asarray(Wv, np.float32), col),
        jax.device_put(np.asarray(bv, np.float32), vec),
        jax.device_put(np.asarray(proj, np.float32), rep),
    )
    out = _gspmd(*args)
    return np.asarray(out, dtype=np.float32)


# ----- fallback: pmap, 2 heads per core -----

def _per_core(X, mask, Wq, bq, Wk, bk, Wv, bv, proj):
    Q = X @ Wq + bq
    K = X @ Wk + bk
    V = X @ Wv + bv

    def split(x):  # [B,S,COLS] -> [B,HPC,S,HD]
        return x.reshape(B, S, HPC, HD).transpose(0, 2, 1, 3)

    out = _attn(split(Q), split(K), split(V), mask, proj)
    return out.transpose(0, 2, 1, 3).reshape(B, S, COLS)


_pmapped = None


def _run_pmap(X, mask, Wq, bq, Wk, bk, Wv, bv, proj):
    global _pmapped
    devs = jax.devices()[:N_CORES]
    if _pmapped is None:
        _pmapped = jax.pmap(_per_core, devices=devs)

    def shard_cols(W):
        return np.stack([np.asarray(W[:, i * COLS:(i + 1) * COLS]) for i in range(N_CORES)])

    def shard_bias(b):
        return np.stack([np.asarray(b[i * COLS:(i + 1) * COLS]) for i in range(N_CORES)])

    rep = lambda a: np.broadcast_to(np.asarray(a), (N_CORES,) + np.asarray(a).shape)

    outs = _pmapped(
        rep(X), rep(mask),
        shard_cols(Wq), shard_bias(bq),
        shard_cols(Wk), shard_bias(bk),
        shard_cols(Wv), shard_bias(bv),
        rep(proj),
    )
    outs = np.asarray(outs)  # [8,B,S,COLS]; core i -> output cols i*128:(i+1)*128
    return np.concatenate(list(outs), axis=-1).astype(np.float32)


_use_gspmd = True


def kernel(X, mask, Wq, bq, Wk, bk, Wv, bv, proj):
    global _use_gspmd
    if _use_gspmd:
        try:
            return _run_gspmd(X, mask, Wq, bq, Wk, bk, Wv, bv, proj)
        except Exception:
            _use_gspmd = False
    return _run_pmap(X, mask, Wq, bq, Wk, bk, Wv, bv, proj)



# revision 14
# speedup vs baseline: 1.3173x; 1.3173x over previous
"""Performer (FAVOR+) attention on 8 trn2 NeuronCores — Bass/Tile kernel.

Sharding: X is S-sharded across cores on the wire (4 MiB bf16/core), then
AllGathered on-device over NeuronLink. Heads are tensor-parallel: each core
owns 2 of the 16 heads (128 columns of the QKV projections) and produces its
128-column slice of the output, returned transposed+bf16 so host assembly is
a cheap dtype cast + view.

Math (validated exact vs reference in numpy):
  exp(u - diag - stab) is computed as raw exp(u) with the per-row factor
  alpha[s] = exp(-diag-stab) folded into the V-side matmul operand, stab
  recovered exactly as max(exp(u)) (monotone), and the +eps / +1e-6
  corrections folded into one K=1 matmul row via beta[s] = eps/alpha_q[s].
"""
import math
import numpy as np
import ml_dtypes

import concourse.bass as bass
import concourse.bacc as bacc
import concourse.mybir as mybir
import concourse.bass_isa as bass_isa
import concourse.tile as tile
import concourse.masks as masks
from concourse.bass_utils import run_bass_kernel_spmd
import jax
from jax.experimental.shard_map import shard_map
from jax.sharding import Mesh, NamedSharding, PartitionSpec
from concurrent.futures import ThreadPoolExecutor

F32 = mybir.dt.float32
BF16 = mybir.dt.bfloat16
AF = mybir.ActivationFunctionType

B, S, D = 4, 4096, 1024
H, HD, M = 16, 64, 256
NC = 8
SC = S // NC            # 512  (S-chunk per core on the wire)
COLS = 128              # output columns per core (2 heads x 64)
NT = S // 128           # 32 s-tiles per batch
EPS = 1e-4
RATIO = M ** -0.5       # 1/16
C_DEN = 1e-6 / (RATIO * RATIO * EPS)   # 2.56
LNEPS = math.log(EPS)
EPS_S = EPS * S

_SEGS = [("xin", B * SC * D), ("wq", D * COLS), ("wk", D * COLS),
         ("wv", D * COLS), ("bqr", COLS), ("bkr", COLS), ("bvr", COLS),
         ("pt2", 128 * M), ("maskb", B * S), ("maskt", 128 * B * NT)]
_OFF = {}
_acc = 0
for _n, _c in _SEGS:
    _OFF[_n] = _acc
    _acc += _c
BLOB_TOT = _acc


def _build():
    nc = bacc.Bacc("TRN2", target_bir_lowering=False, debug=False,
                   enable_asserts=False, num_devices=NC)

    blob = nc.dram_tensor("blob", [BLOB_TOT], BF16, kind="ExternalInput")
    _o = {}

    def _seg(name, n):
        _o[name] = (_OFF[name], n)
        a, b_ = _OFF[name], _OFF[name] + n
        return blob.ap()[a:b_]

    xin = _seg("xin", B * SC * D).rearrange("(r c) -> r c", c=D)
    wq = _seg("wq", D * COLS).rearrange("(r c) -> r c", c=COLS)
    wk = _seg("wk", D * COLS).rearrange("(r c) -> r c", c=COLS)
    wv = _seg("wv", D * COLS).rearrange("(r c) -> r c", c=COLS)
    bqr = _seg("bqr", COLS).rearrange("(r c) -> r c", c=COLS)
    bkr = _seg("bkr", COLS).rearrange("(r c) -> r c", c=COLS)
    bvr = _seg("bvr", COLS).rearrange("(r c) -> r c", c=COLS)
    pt2 = _seg("pt2", 128 * M).rearrange("(r c) -> r c", c=M)
    maskb_d = _seg("maskb", B * S).rearrange("(r c) -> r c", c=S)
    maskt_d = _seg("maskt", 128 * B * NT).rearrange("(r c) -> r c", c=B * NT)
    yout = nc.dram_tensor("yout", [COLS, B * S], BF16, kind="ExternalOutput")

    with tile.TileContext(nc) as tc:
        with tc.tile_pool(name="dram", bufs=1, space="DRAM") as dram, \
             tc.tile_pool(name="drs", bufs=2, space="DRAM") as drs, \
             tc.tile_pool(name="const", bufs=1) as cpool, \
             tc.tile_pool(name="big", bufs=1) as big, \
             tc.tile_pool(name="work", bufs=2) as work, \
             tc.tile_pool(name="psp", bufs=8, space="PSUM") as psp:

            # ---- constants ----
            w_sb = {}
            for name, w in (("q", wq), ("k", wk), ("v", wv)):
                t = cpool.tile([128, D], BF16, name=f"w{name}_sb")
                for kk in range(8):
                    nc.sync.dma_start(t[:, kk * 128:(kk + 1) * 128],
                                      w[kk * 128:(kk + 1) * 128, :])
                w_sb[name] = t
            pt2_sb = cpool.tile([128, M], BF16, name="pt2_sb")
            nc.sync.dma_start(pt2_sb[:], pt2)
            b_sb = {}
            for name, bb in (("q", bqr), ("k", bkr), ("v", bvr)):
                t = cpool.tile([1, COLS], BF16, name=f"b{name}_sb")
                nc.sync.dma_start(t[:], bb)
                b_sb[name] = t
            maskt_b = cpool.tile([128, B * NT], BF16, name="maskt_b")
            nc.sync.dma_start(maskt_b[:], maskt_d)
            maskt = cpool.tile([128, B * NT], F32, name="maskt")
            nc.vector.tensor_copy(maskt[:], maskt_b[:])
            ident = cpool.tile([128, 128], F32, name="ident")
            masks.make_identity(nc, ident[:])
            ones_row = cpool.tile([1, 512], BF16, name="ones_row")
            nc.vector.memset(ones_row[:], 1.0)
            onecol = cpool.tile([128, 1], BF16, name="onecol")
            nc.vector.memset(onecol[:], 1.0)
            headmask = cpool.tile([128, 2], BF16, name="headmask")
            nc.vector.memset(headmask[:], 0.0)
            nc.vector.memset(headmask[0:64, 0:1], 1.0 / 128.0)
            nc.vector.memset(headmask[64:128, 1:2], 1.0 / 128.0)
            lneps = cpool.tile([128, 1], F32, name="lneps")
            nc.vector.memset(lneps[:], LNEPS)

            # ---- allgather X ----
            bounce = dram.tile([B * SC, D], BF16)
            xg = dram.tile([NC * B * SC, D], BF16, addr_space="Shared")
            nc.sync.dma_start(bounce[:], xin)
            nc.gpsimd.collective_compute(
                "AllGather", mybir.AluOpType.bypass,
                replica_groups=[list(range(NC))],
                ins=[bounce.opt()], outs=[xg.opt()])

            for b in range(B):
                # ---- mask row (for K masking) ----
                mrowb = big.tile([1, S], BF16, name="mrowb")
                nc.sync.dma_start(mrowb[:], maskb_d[b:b + 1, :])
                mask_bc = big.tile([128, S], BF16, name="mask_bc")
                nc.gpsimd.partition_broadcast(mask_bc[:], mrowb[:],
                                              channels=128)
                mtb = maskt[:, b * NT:(b + 1) * NT]

                qt = big.tile([128, S], BF16, name="qt")
                kt = big.tile([128, S], BF16, name="kt")
                vs = big.tile([128, S], BF16, name="vs")
                dts = {"q": [], "k": []}
                for name in ("q", "k"):
                    for hh in range(2):
                        dts[name].append(work.tile(
                            [128, NT], F32, name=f"d{name}{hh}_t", bufs=1))

                for n in range(8):
                    # X^T chunk for s in [n*512, (n+1)*512) == core n's block
                    xtn = work.tile([128, 8 * 512], BF16, name="xtn")
                    for kk in range(8):
                        nc.sync.dma_start_transpose(
                            xtn[:, kk * 512:(kk + 1) * 512],
                            xg[(n * B + b) * SC:(n * B + b + 1) * SC,
                               kk * 128:(kk + 1) * 128])

                    for name, dst in (("q", qt), ("k", kt)):
                        ps = psp.tile([128, 512], F32, tag="ps", bufs=4)
                        for kk in range(8):
                            nc.tensor.matmul(
                                ps[:], w_sb[name][:, kk * 128:(kk + 1) * 128],
                                xtn[:, kk * 512:(kk + 1) * 512],
                                start=(kk == 0), stop=False)
                        nc.tensor.matmul(ps[:], b_sb[name][:], ones_row[:],
                                         start=False, stop=True)
                        sl = dst[:, n * 512:(n + 1) * 512]
                        if name == "q":
                            nc.scalar.activation(sl, ps[:], AF.Copy)
                        else:
                            nc.vector.tensor_mul(
                                sl, ps[:], mask_bc[:, n * 512:(n + 1) * 512])

                        # diag: square chunk, headmask matmul, stage transposed
                        sqc = work.tile([128, 512], BF16, name="sqc")
                        nc.vector.tensor_mul(sqc[:], sl, sl)
                        psd = psp.tile([128, 512], F32, tag="ps", bufs=4)
                        pd = psd[0:2, :]
                        nc.tensor.matmul(pd, headmask[:], sqc[:],
                                         start=True, stop=True)
                        dstg = work.tile([2, 512], F32, name="dstg")
                        nc.any.tensor_copy(dstg[:], pd)
                        dstg_d = drs.tile([1024], F32, name="dstg_d")
                        nc.sync.dma_start(
                            dstg_d.rearrange("(h s) -> h s", h=2), dstg[:])
                        for hh in range(2):
                            nc.sync.dma_start(
                                dts[name][hh][:, n * 4:(n + 1) * 4],
                                dstg_d[hh * 512:(hh + 1) * 512].rearrange(
                                    "(t p) -> p t", p=128))

                    # V tiles for this n-chunk (4 s-tiles)
                    for ti in range(4):
                        t = n * 4 + ti
                        psv = psp.tile([128, 512], F32, tag="ps", bufs=4)
                        pv = psv[:, 0:128]
                        for kk in range(8):
                            nc.tensor.matmul(
                                pv,
                                xtn[:, kk * 512 + ti * 128:
                                    kk * 512 + (ti + 1) * 128],
                                w_sb["v"][:, kk * 128:(kk + 1) * 128],
                                start=(kk == 0), stop=False)
                        nc.tensor.matmul(pv, ones_row[:, 0:128], b_sb["v"][:],
                                         start=False, stop=True)
                        nc.vector.tensor_scalar_mul(
                            vs[:, t * 128:(t + 1) * 128], pv, mtb[:, t:t + 1])

                # ---- SV (both heads) ----
                ps_sv = psp.tile([128, 512], F32, tag="ps", bufs=4)
                psv = ps_sv[:, 0:1]
                for t in range(NT):
                    nc.tensor.matmul(psv, vs[:, t * 128:(t + 1) * 128],
                                     onecol[:], start=(t == 0),
                                     stop=(t == NT - 1))
                sv_eps = work.tile([128, 1], F32, name="sv_eps")
                nc.vector.tensor_scalar_mul(sv_eps[:], psv, EPS)

                for hh in range(2):
                    hsl = slice(hh * 64, hh * 64 + 64)

                    # ---- E_k = exp(u_k)  [s, m] ----
                    ek = big.tile([128, NT * M], BF16, name="ek")
                    for t4 in range(NT // 4):
                        ps = psp.tile([128, 4 * M], F32, tag="ps4", bufs=2)
                        for i in range(4):
                            t = t4 * 4 + i
                            nc.tensor.matmul(
                                ps[:, i * M:(i + 1) * M],
                                kt[hsl, t * 128:(t + 1) * 128],
                                pt2_sb[hsl, :], start=True, stop=True)
                        nc.scalar.activation(
                            ek[:, t4 * 4 * M:(t4 + 1) * 4 * M], ps[:], AF.Exp)

                    # ---- alpha_k ----
                    mx = work.tile([128, 1], F32, name="mx")
                    nc.vector.reduce_max(out=mx[:], in_=ek[:],
                                         axis=mybir.AxisListType.X)
                    mek = work.tile([128, 1], F32, name="mek")
                    nc.gpsimd.partition_all_reduce(
                        mek[:], mx[:], channels=128,
                        reduce_op=bass_isa.ReduceOp.max)
                    rmek = work.tile([128, 1], F32, name="rmek")
                    nc.vector.reciprocal(rmek[:], mek[:])
                    ak_t = work.tile([128, NT], F32, name="ak_t")
                    nc.scalar.activation(ak_t[:], dts["k"][hh][:], AF.Exp,
                                         scale=-1.0)
                    nc.vector.tensor_scalar_mul(ak_t[:], ak_t[:], rmek[:])

                    # ---- Vaug [s, 65] tiles ----
                    vaug = big.tile([128, NT * 65], BF16, name="vaug")
                    for t in range(NT):
                        nc.vector.tensor_scalar_mul(
                            vaug[:, t * 65: t * 65 + 64],
                            vs[:, t * 128 + hh * 64: t * 128 + hh * 64 + 64],
                            ak_t[:, t:t + 1])
                        nc.vector.tensor_copy(
                            vaug[:, t * 65 + 64: t * 65 + 65], ak_t[:, t:t + 1])

                    # ---- kvaug^T [65, m] ----
                    ps_kv = psp.tile([128, 512], F32, tag="ps", bufs=4)
                    pkv = ps_kv[0:65, 0:M]
                    for t in range(NT):
                        nc.tensor.matmul(pkv, vaug[:, t * 65:(t + 1) * 65],
                                         ek[:, t * M:(t + 1) * M],
                                         start=(t == 0), stop=(t == NT - 1))
                    w2t = work.tile([65, M], F32, name="w2t")
                    nc.any.tensor_copy(w2t[:], pkv)
                    nc.vector.tensor_scalar_add(w2t[0:64, :], w2t[0:64, :],
                                                sv_eps[hsl, :])
                    nc.vector.tensor_scalar_add(w2t[64:65, :], w2t[64:65, :],
                                                EPS_S)

                    # K1aug
                    k1 = work.tile([65, 1], F32, name="k1")
                    nc.vector.reduce_sum(out=k1[:], in_=w2t[:],
                                         axis=mybir.AxisListType.X)
                    nc.vector.tensor_scalar_add(k1[64:65, :], k1[64:65, :],
                                                C_DEN)
                    k1b = work.tile([65, 1], BF16, name="k1b")
                    nc.vector.tensor_copy(k1b[:], k1[:])
                    k1_d = drs.tile([65], BF16, name="k1_d")
                    nc.sync.dma_start(
                        k1_d.rearrange("(p a) -> p a", a=1), k1b[:])
                    k1row = work.tile([1, 65], BF16, name="k1row")
                    nc.sync.dma_start(
                        k1row[:], k1_d.rearrange("(a p) -> a p", a=1))

                    # W2 [m, 65] bf16 (2 chunks, PE transpose)
                    w2 = []
                    for mc in range(2):
                        ps_t = psp.tile([128, 512], F32, tag="ps", bufs=4)
                        pt_ = ps_t[:, 0:65]
                        nc.tensor.transpose(
                            pt_, w2t[:, mc * 128:(mc + 1) * 128],
                            ident[0:65, 0:65])
                        wsb = work.tile([128, 65], BF16, name=f"w2_{mc}")
                        nc.any.tensor_copy(wsb[:], pt_)
                        w2.append(wsb)

                    # ---- E_q = exp(u_q)  [m, s] (2 chunks) ----
                    eq = []
                    for mc in range(2):
                        eqc = big.tile([128, S], BF16, name=f"eq{mc}")
                        eq.append(eqc)
                        for n2 in range(4):
                            ps = psp.tile([128, 2 * 512], F32, tag="ps4", bufs=2)
                            for i in range(2):
                                n = n2 * 2 + i
                                nc.tensor.matmul(
                                    ps[:, i * 512:(i + 1) * 512],
                                    pt2_sb[hsl, mc * 128:(mc + 1) * 128],
                                    qt[hsl, n * 512:(n + 1) * 512],
                                    start=True, stop=True)
                            nc.scalar.activation(
                                eqc[:, n2 * 1024:(n2 + 1) * 1024], ps[:],
                                AF.Exp)

                    # ---- beta row ----
                    mq2 = big.tile([128, S], BF16, name="mq2")
                    nc.vector.tensor_max(mq2[:], eq[0][:], eq[1][:])
                    prs = big.tile([128, S], BF16, name="prs")
                    nc.gpsimd.partition_all_reduce(
                        prs[:], mq2[:], channels=128,
                        reduce_op=bass_isa.ReduceOp.max)
                    mq_t = work.tile([128, NT], BF16, name="mq_t")
                    mq_d = drs.tile([S], BF16, name="mq_d")
                    nc.sync.dma_start(
                        mq_d.rearrange("(a s) -> a s", a=1), prs[0:1, :])
                    nc.sync.dma_start(
                        mq_t[:], mq_d.rearrange("(t p) -> p t", p=128))
                    ebq = work.tile([128, NT], F32, name="ebq")
                    nc.scalar.activation(ebq[:], dts["q"][hh][:], AF.Exp,
                                         bias=lneps[:])
                    beta_t = work.tile([128, NT], BF16, name="beta_t")
                    nc.vector.tensor_mul(beta_t[:], ebq[:], mq_t[:])
                    bt_d = drs.tile([S], BF16, name="bt_d")
                    nc.sync.dma_start(
                        bt_d.rearrange("(t p) -> p t", p=128), beta_t[:])
                    brow = work.tile([1, S], BF16, name="brow", bufs=1)
                    nc.sync.dma_start(
                        brow[:], bt_d.rearrange("(a s) -> a s", a=1))

                    # ---- numden + divide + out ----
                    oh = big.tile([64, S], BF16, name="oh")
                    for n in range(8):
                        ps_nd = psp.tile([128, 512], F32, tag="ps", bufs=4)
                        pnd = ps_nd[0:65, :]
                        nc.tensor.matmul(pnd, w2[0][:],
                                         eq[0][:, n * 512:(n + 1) * 512],
                                         start=True, stop=False)
                        nc.tensor.matmul(pnd, w2[1][:],
                                         eq[1][:, n * 512:(n + 1) * 512],
                                         start=False, stop=False)
                        nc.tensor.matmul(pnd, k1row[:],
                                         brow[:, n * 512:(n + 1) * 512],
                                         start=False, stop=True)
                        rec = work.tile([1, 512], F32, name="rec")
                        nc.vector.reciprocal(rec[:], pnd[64:65, :])
                        recb = work.tile([64, 512], F32, name="recb")
                        nc.gpsimd.partition_broadcast(recb[:], rec[:],
                                                      channels=64)
                        nc.vector.tensor_mul(oh[:, n * 512:(n + 1) * 512],
                                             pnd[0:64, :], recb[:])
                    nc.sync.dma_start(
                        yout.ap()[hsl, b * S:(b + 1) * S], oh[:])

    nc.compile()
    return nc


_runner = None


class _Runner:
    """Cached jit over the bass custom call (mirrors run_bass_via_pjrt's
    multi-core path, built once). Outputs are donated from the previous
    call's output buffer — the kernel writes every output element."""

    def __init__(self, nc):
        from concourse import bass2jax as b2j
        b2j.install_neuronx_cc_hook()
        self.nc = nc
        fn = nc.m.functions[0]
        pname = nc.partition_id_tensor.name if nc.partition_id_tensor else None
        in_names, out_names, out_avals = [], [], []
        for alloc in fn.allocations:
            if not isinstance(alloc, mybir.MemoryLocationSet):
                continue
            name = alloc.memorylocations[0].name
            if alloc.kind == "ExternalInput":
                if name != pname:
                    in_names.append(name)
            elif alloc.kind == "ExternalOutput":
                assert alloc.tensor_shape is not None
                out_names.append(name)
                out_avals.append(jax.core.ShapedArray(
                    tuple(alloc.tensor_shape), mybir.dt.np(alloc.dtype)))
        self.in_names = list(in_names)
        self.out_names = out_names
        self.out_avals = out_avals
        n_params = len(in_names)
        all_in = tuple(in_names + out_names + ([pname] if pname else []))
        donate = tuple(range(n_params, n_params + len(out_names)))

        def _body(*args):
            operands = list(args)
            if pname is not None:
                operands.append(b2j.partition_id_tensor())
            outs = b2j._bass_exec_p.bind(
                *operands,
                out_avals=tuple(out_avals),
                in_names=all_in,
                out_names=tuple(out_names),
                lowering_input_output_aliases=(),
                sim_require_finite=True,
                sim_require_nnan=True,
                nc=nc,
            )
            return tuple(outs)

        devices = jax.devices()[:NC]
        self.mesh = Mesh(np.asarray(devices), ("core",))
        nin = n_params + len(out_names)
        self.sharding = NamedSharding(self.mesh, PartitionSpec("core"))
        self.jit = jax.jit(
            shard_map(_body, mesh=self.mesh,
                      in_specs=(PartitionSpec("core"),) * nin,
                      out_specs=(PartitionSpec("core"),) * len(out_names),
                      check_rep=False),
            donate_argnums=donate, keep_unused=True)
        self._zeros = None

    def _make_zeros(self):
        outs = []
        for av in self.out_avals:
            shape = (NC * av.shape[0],) + av.shape[1:]
            z = jax.jit(lambda s=shape, d=av.dtype: jax.numpy.zeros(s, d),
                        out_shardings=self.sharding)()
            outs.append(z)
        return outs

    def run(self, per_core_blobs):
        if self._zeros is None:
            self._zeros = self._make_zeros()
        g_in = np.concatenate(per_core_blobs, axis=0)
        outs = self.jit(g_in, *self._zeros)
        # recycle output buffers as next call's donated outputs: invalid
        # (donated consumed). Rebuild zeros lazily from outs? outs are the
        # new buffers; they can be donated next call (values overwritten).
        self._zeros = list(outs)
        arr = outs[0]
        shards = sorted(arr.addressable_shards,
                        key=lambda sh: (sh.index[0].start or 0))
        with ThreadPoolExecutor(NC) as ex:
            datas = list(ex.map(lambda sh: np.asarray(sh.data), shards))
        return datas


def _get_runner():
    global _runner
    if _runner is None:
        _runner = _Runner(_build())
    return _runner


def kernel(X, mask, Wq, bq, Wk, bk, Wv, bv, proj):
    r = _get_runner()
    in_maps = _prep_inputs(X, mask, Wq, bq, Wk, bk, Wv, bv, proj)
    datas = r.run([m["blob"] for m in in_maps])
    A = np.concatenate(datas, axis=0).astype(np.float32)   # [D, B*S]
    return A.reshape(D, B, S).transpose(1, 2, 0)


# revision 16
# speedup vs baseline: 4.8366x; 3.6717x over previous
"""Performer (FAVOR+) attention on 8 trn2 NeuronCores — Bass/Tile kernel.

Sharding: X is S-sharded across cores on the wire (4 MiB bf16/core), then
AllGathered on-device over NeuronLink. Heads are tensor-parallel: each core
owns 2 of the 16 heads (128 columns of the QKV projections) and produces its
128-column slice of the output, returned transposed+bf16 so host assembly is
a cheap dtype cast + view.

Math (validated exact vs reference in numpy):
  exp(u - diag - stab) is computed as raw exp(u) with the per-row factor
  alpha[s] = exp(-diag-stab) folded into the V-side matmul operand, stab
  recovered exactly as max(exp(u)) (monotone), and the +eps / +1e-6
  corrections folded into one K=1 matmul row via beta[s] = eps/alpha_q[s].
"""
import math
import numpy as np
import ml_dtypes

import concourse.bass as bass
import concourse.bacc as bacc
import concourse.mybir as mybir
import concourse.bass_isa as bass_isa
import concourse.tile as tile
import concourse.masks as masks
from concourse.bass_utils import run_bass_kernel_spmd
import jax
from jax.experimental.shard_map import shard_map
from jax.sharding import Mesh, NamedSharding, PartitionSpec
from concurrent.futures import ThreadPoolExecutor

F32 = mybir.dt.float32
BF16 = mybir.dt.bfloat16
AF = mybir.ActivationFunctionType

B, S, D = 4, 4096, 1024
H, HD, M = 16, 64, 256
NC = 8
SC = S // NC            # 512  (S-chunk per core on the wire)
COLS = 128              # output columns per core (2 heads x 64)
NT = S // 128           # 32 s-tiles per batch
EPS = 1e-4
RATIO = M ** -0.5       # 1/16
C_DEN = 1e-6 / (RATIO * RATIO * EPS)   # 2.56
LNEPS = math.log(EPS)
EPS_S = EPS * S

_SEGS = [("xin", B * SC * D), ("wq", D * COLS), ("wk", D * COLS),
         ("wv", D * COLS), ("bqr", COLS), ("bkr", COLS), ("bvr", COLS),
         ("pt2", 128 * M), ("maskb", B * S), ("maskt", 128 * B * NT)]
_OFF = {}
_acc = 0
for _n, _c in _SEGS:
    _OFF[_n] = _acc
    _acc += _c
BLOB_TOT = _acc


def _build():
    nc = bacc.Bacc("TRN2", target_bir_lowering=False, debug=False,
                   enable_asserts=False, num_devices=NC)

    blob = nc.dram_tensor("blob", [BLOB_TOT], BF16, kind="ExternalInput")
    _o = {}

    def _seg(name, n):
        _o[name] = (_OFF[name], n)
        a, b_ = _OFF[name], _OFF[name] + n
        return blob.ap()[a:b_]

    xin = _seg("xin", B * SC * D).rearrange("(r c) -> r c", c=D)
    wq = _seg("wq", D * COLS).rearrange("(r c) -> r c", c=COLS)
    wk = _seg("wk", D * COLS).rearrange("(r c) -> r c", c=COLS)
    wv = _seg("wv", D * COLS).rearrange("(r c) -> r c", c=COLS)
    bqr = _seg("bqr", COLS).rearrange("(r c) -> r c", c=COLS)
    bkr = _seg("bkr", COLS).rearrange("(r c) -> r c", c=COLS)
    bvr = _seg("bvr", COLS).rearrange("(r c) -> r c", c=COLS)
    pt2 = _seg("pt2", 128 * M).rearrange("(r c) -> r c", c=M)
    maskb_d = _seg("maskb", B * S).rearrange("(r c) -> r c", c=S)
    maskt_d = _seg("maskt", 128 * B * NT).rearrange("(r c) -> r c", c=B * NT)
    yout = nc.dram_tensor("yout", [COLS, B * S], BF16, kind="ExternalOutput")

    with tile.TileContext(nc) as tc:
        with tc.tile_pool(name="dram", bufs=1, space="DRAM") as dram, \
             tc.tile_pool(name="drs", bufs=2, space="DRAM") as drs, \
             tc.tile_pool(name="const", bufs=1) as cpool, \
             tc.tile_pool(name="big", bufs=1) as big, \
             tc.tile_pool(name="work", bufs=2) as work, \
             tc.tile_pool(name="psp", bufs=8, space="PSUM") as psp:

            # ---- constants ----
            w_sb = {}
            for name, w in (("q", wq), ("k", wk), ("v", wv)):
                t = cpool.tile([128, D], BF16, name=f"w{name}_sb")
                for kk in range(8):
                    nc.sync.dma_start(t[:, kk * 128:(kk + 1) * 128],
                                      w[kk * 128:(kk + 1) * 128, :])
                w_sb[name] = t
            pt2_sb = cpool.tile([128, M], BF16, name="pt2_sb")
            nc.sync.dma_start(pt2_sb[:], pt2)
            b_sb = {}
            for name, bb in (("q", bqr), ("k", bkr), ("v", bvr)):
                t = cpool.tile([1, COLS], BF16, name=f"b{name}_sb")
                nc.sync.dma_start(t[:], bb)
                b_sb[name] = t
            maskt_b = cpool.tile([128, B * NT], BF16, name="maskt_b")
            nc.sync.dma_start(maskt_b[:], maskt_d)
            maskt = cpool.tile([128, B * NT], F32, name="maskt")
            nc.vector.tensor_copy(maskt[:], maskt_b[:])
            ident = cpool.tile([128, 128], F32, name="ident")
            masks.make_identity(nc, ident[:])
            ones_row = cpool.tile([1, 512], BF16, name="ones_row")
            nc.vector.memset(ones_row[:], 1.0)
            onecol = cpool.tile([128, 1], BF16, name="onecol")
            nc.vector.memset(onecol[:], 1.0)
            headmask = cpool.tile([128, 2], BF16, name="headmask")
            nc.vector.memset(headmask[:], 0.0)
            nc.vector.memset(headmask[0:64, 0:1], 1.0 / 128.0)
            nc.vector.memset(headmask[64:128, 1:2], 1.0 / 128.0)
            lneps = cpool.tile([128, 1], F32, name="lneps")
            nc.vector.memset(lneps[:], LNEPS)

            # ---- allgather X ----
            bounce = dram.tile([B * SC, D], BF16)
            xg = dram.tile([NC * B * SC, D], BF16, addr_space="Shared")
            nc.sync.dma_start(bounce[:], xin)
            nc.gpsimd.collective_compute(
                "AllGather", mybir.AluOpType.bypass,
                replica_groups=[list(range(NC))],
                ins=[bounce.opt()], outs=[xg.opt()])

            for b in range(B):
                # ---- mask row (for K masking) ----
                mrowb = big.tile([1, S], BF16, name="mrowb")
                nc.sync.dma_start(mrowb[:], maskb_d[b:b + 1, :])
                mask_bc = big.tile([128, S], BF16, name="mask_bc")
                nc.gpsimd.partition_broadcast(mask_bc[:], mrowb[:],
                                              channels=128)
                mtb = maskt[:, b * NT:(b + 1) * NT]

                qt = big.tile([128, S], BF16, name="qt")
                kt = big.tile([128, S], BF16, name="kt")
                vs = big.tile([128, S], BF16, name="vs")
                dts = {"q": [], "k": []}
                for name in ("q", "k"):
                    for hh in range(2):
                        dts[name].append(work.tile(
                            [128, NT], F32, name=f"d{name}{hh}_t", bufs=1))

                for n in range(8):
                    # X^T chunk for s in [n*512, (n+1)*512) == core n's block
                    xtn = work.tile([128, 8 * 512], BF16, name="xtn")
                    for kk in range(8):
                        nc.sync.dma_start_transpose(
                            xtn[:, kk * 512:(kk + 1) * 512],
                            xg[(n * B + b) * SC:(n * B + b + 1) * SC,
                               kk * 128:(kk + 1) * 128])

                    for name, dst in (("q", qt), ("k", kt)):
                        ps = psp.tile([128, 512], F32, tag="ps", bufs=4)
                        for kk in range(8):
                            nc.tensor.matmul(
                                ps[:], w_sb[name][:, kk * 128:(kk + 1) * 128],
                                xtn[:, kk * 512:(kk + 1) * 512],
                                start=(kk == 0), stop=False)
                        nc.tensor.matmul(ps[:], b_sb[name][:], ones_row[:],
                                         start=False, stop=True)
                        sl = dst[:, n * 512:(n + 1) * 512]
                        if name == "q":
                            nc.scalar.activation(sl, ps[:], AF.Copy)
                        else:
                            nc.vector.tensor_mul(
                                sl, ps[:], mask_bc[:, n * 512:(n + 1) * 512])

                        # diag: square chunk, headmask matmul, stage transposed
                        sqc = work.tile([128, 512], BF16, name="sqc")
                        nc.vector.tensor_mul(sqc[:], sl, sl)
                        psd = psp.tile([128, 512], F32, tag="ps", bufs=4)
                        pd = psd[0:2, :]
                        nc.tensor.matmul(pd, headmask[:], sqc[:],
                                         start=True, stop=True)
                        dstg = work.tile([2, 512], F32, name="dstg")
                        nc.any.tensor_copy(dstg[:], pd)
                        dstg_d = drs.tile([1024], F32, name="dstg_d")
                        nc.sync.dma_start(
                            dstg_d.rearrange("(h s) -> h s", h=2), dstg[:])
                        for hh in range(2):
                            nc.sync.dma_start(
                                dts[name][hh][:, n * 4:(n + 1) * 4],
                                dstg_d[hh * 512:(hh + 1) * 512].rearrange(
                                    "(t p) -> p t", p=128))

                    # V tiles for this n-chunk (4 s-tiles)
                    for ti in range(4):
                        t = n * 4 + ti
                        psv = psp.tile([128, 512], F32, tag="ps", bufs=4)
                        pv = psv[:, 0:128]
                        for kk in range(8):
                            nc.tensor.matmul(
                                pv,
                                xtn[:, kk * 512 + ti * 128:
                                    kk * 512 + (ti + 1) * 128],
                                w_sb["v"][:, kk * 128:(kk + 1) * 128],
                                start=(kk == 0), stop=False)
                        nc.tensor.matmul(pv, ones_row[:, 0:128], b_sb["v"][:],
                                         start=False, stop=True)
                        nc.vector.tensor_scalar_mul(
                            vs[:, t * 128:(t + 1) * 128], pv, mtb[:, t:t + 1])

                # ---- SV (both heads) ----
                ps_sv = psp.tile([128, 512], F32, tag="ps", bufs=4)
                psv = ps_sv[:, 0:1]
                for t in range(NT):
                    nc.tensor.matmul(psv, vs[:, t * 128:(t + 1) * 128],
                                     onecol[:], start=(t == 0),
                                     stop=(t == NT - 1))
                sv_eps = work.tile([128, 1], F32, name="sv_eps")
                nc.vector.tensor_scalar_mul(sv_eps[:], psv, EPS)

                for hh in range(2):
                    hsl = slice(hh * 64, hh * 64 + 64)

                    # ---- E_k = exp(u_k)  [s, m] ----
                    ek = big.tile([128, NT * M], BF16, name="ek")
                    for t4 in range(NT // 4):
                        ps = psp.tile([128, 4 * M], F32, tag="ps4", bufs=2)
                        for i in range(4):
                            t = t4 * 4 + i
                            nc.tensor.matmul(
                                ps[:, i * M:(i + 1) * M],
                                kt[hsl, t * 128:(t + 1) * 128],
                                pt2_sb[hsl, :], start=True, stop=True)
                        nc.scalar.activation(
                            ek[:, t4 * 4 * M:(t4 + 1) * 4 * M], ps[:], AF.Exp)

                    # ---- alpha_k ----
                    mx = work.tile([128, 1], F32, name="mx")
                    nc.vector.reduce_max(out=mx[:], in_=ek[:],
                                         axis=mybir.AxisListType.X)
                    mek = work.tile([128, 1], F32, name="mek")
                    nc.gpsimd.partition_all_reduce(
                        mek[:], mx[:], channels=128,
                        reduce_op=bass_isa.ReduceOp.max)
                    rmek = work.tile([128, 1], F32, name="rmek")
                    nc.vector.reciprocal(rmek[:], mek[:])
                    ak_t = work.tile([128, NT], F32, name="ak_t")
                    nc.scalar.activation(ak_t[:], dts["k"][hh][:], AF.Exp,
                                         scale=-1.0)
                    nc.vector.tensor_scalar_mul(ak_t[:], ak_t[:], rmek[:])

                    # ---- Vaug [s, 65] tiles ----
                    vaug = big.tile([128, NT * 65], BF16, name="vaug")
                    for t in range(NT):
                        nc.vector.tensor_scalar_mul(
                            vaug[:, t * 65: t * 65 + 64],
                            vs[:, t * 128 + hh * 64: t * 128 + hh * 64 + 64],
                            ak_t[:, t:t + 1])
                        nc.vector.tensor_copy(
                            vaug[:, t * 65 + 64: t * 65 + 65], ak_t[:, t:t + 1])

                    # ---- kvaug^T [65, m] ----
                    ps_kv = psp.tile([128, 512], F32, tag="ps", bufs=4)
                    pkv = ps_kv[0:65, 0:M]
                    for t in range(NT):
                        nc.tensor.matmul(pkv, vaug[:, t * 65:(t + 1) * 65],
                                         ek[:, t * M:(t + 1) * M],
                                         start=(t == 0), stop=(t == NT - 1))
                    w2t = work.tile([65, M], F32, name="w2t")
                    nc.any.tensor_copy(w2t[:], pkv)
                    nc.vector.tensor_scalar_add(w2t[0:64, :], w2t[0:64, :],
                                                sv_eps[hsl, :])
                    nc.vector.tensor_scalar_add(w2t[64:65, :], w2t[64:65, :],
                                                EPS_S)

                    # K1aug
                    k1 = work.tile([65, 1], F32, name="k1")
                    nc.vector.reduce_sum(out=k1[:], in_=w2t[:],
                                         axis=mybir.AxisListType.X)
                    nc.vector.tensor_scalar_add(k1[64:65, :], k1[64:65, :],
                                                C_DEN)
                    k1b = work.tile([65, 1], BF16, name="k1b")
                    nc.vector.tensor_copy(k1b[:], k1[:])
                    k1_d = drs.tile([65], BF16, name="k1_d")
                    nc.sync.dma_start(
                        k1_d.rearrange("(p a) -> p a", a=1), k1b[:])
                    k1row = work.tile([1, 65], BF16, name="k1row")
                    nc.sync.dma_start(
                        k1row[:], k1_d.rearrange("(a p) -> a p", a=1))

                    # W2 [m, 65] bf16 (2 chunks, PE transpose)
                    w2 = []
                    for mc in range(2):
                        ps_t = psp.tile([128, 512], F32, tag="ps", bufs=4)
                        pt_ = ps_t[:, 0:65]
                        nc.tensor.transpose(
                            pt_, w2t[:, mc * 128:(mc + 1) * 128],
                            ident[0:65, 0:65])
                        wsb = work.tile([128, 65], BF16, name=f"w2_{mc}")
                        nc.any.tensor_copy(wsb[:], pt_)
                        w2.append(wsb)

                    # ---- E_q = exp(u_q)  [m, s] (2 chunks) ----
                    eq = []
                    for mc in range(2):
                        eqc = big.tile([128, S], BF16, name=f"eq{mc}")
                        eq.append(eqc)
                        for n2 in range(4):
                            ps = psp.tile([128, 2 * 512], F32, tag="ps4", bufs=2)
                            for i in range(2):
                                n = n2 * 2 + i
                                nc.tensor.matmul(
                                    ps[:, i * 512:(i + 1) * 512],
                                    pt2_sb[hsl, mc * 128:(mc + 1) * 128],
                                    qt[hsl, n * 512:(n + 1) * 512],
                                    start=True, stop=True)
                            nc.scalar.activation(
                                eqc[:, n2 * 1024:(n2 + 1) * 1024], ps[:],
                                AF.Exp)

                    # ---- beta row ----
                    mq2 = big.tile([128, S], BF16, name="mq2")
                    nc.vector.tensor_max(mq2[:], eq[0][:], eq[1][:])
                    prs = big.tile([128, S], BF16, name="prs")
                    nc.gpsimd.partition_all_reduce(
                        prs[:], mq2[:], channels=128,
                        reduce_op=bass_isa.ReduceOp.max)
                    mq_t = work.tile([128, NT], BF16, name="mq_t")
                    mq_d = drs.tile([S], BF16, name="mq_d")
                    nc.sync.dma_start(
                        mq_d.rearrange("(a s) -> a s", a=1), prs[0:1, :])
                    nc.sync.dma_start(
                        mq_t[:], mq_d.rearrange("(t p) -> p t", p=128))
                    ebq = work.tile([128, NT], F32, name="ebq")
                    nc.scalar.activation(ebq[:], dts["q"][hh][:], AF.Exp,
                                         bias=lneps[:])
                    beta_t = work.tile([128, NT], BF16, name="beta_t")
                    nc.vector.tensor_mul(beta_t[:], ebq[:], mq_t[:])
                    bt_d = drs.tile([S], BF16, name="bt_d")
                    nc.sync.dma_start(
                        bt_d.rearrange("(t p) -> p t", p=128), beta_t[:])
                    brow = work.tile([1, S], BF16, name="brow", bufs=1)
                    nc.sync.dma_start(
                        brow[:], bt_d.rearrange("(a s) -> a s", a=1))

                    # ---- numden + divide + out ----
                    oh = big.tile([64, S], BF16, name="oh")
                    for n in range(8):
                        ps_nd = psp.tile([128, 512], F32, tag="ps", bufs=4)
                        pnd = ps_nd[0:65, :]
                        nc.tensor.matmul(pnd, w2[0][:],
                                         eq[0][:, n * 512:(n + 1) * 512],
                                         start=True, stop=False)
                        nc.tensor.matmul(pnd, w2[1][:],
                                         eq[1][:, n * 512:(n + 1) * 512],
                                         start=False, stop=False)
                        nc.tensor.matmul(pnd, k1row[:],
                                         brow[:, n * 512:(n + 1) * 512],
                                         start=False, stop=True)
                        rec = work.tile([1, 512], F32, name="rec")
                        nc.vector.reciprocal(rec[:], pnd[64:65, :])
                        recb = work.tile([64, 512], F32, name="recb")
                        nc.gpsimd.partition_broadcast(recb[:], rec[:],
                                                      channels=64)
                        nc.vector.tensor_mul(oh[:, n * 512:(n + 1) * 512],
                                             pnd[0:64, :], recb[:])
                    nc.sync.dma_start(
                        yout.ap()[hsl, b * S:(b + 1) * S], oh[:])

    nc.compile()
    return nc


_runner = None


def _prep_inputs(X, mask, Wq, bq, Wk, bk, Wv, bv, proj):
    bf = ml_dtypes.bfloat16
    Xb = np.asarray(X, np.float32).astype(bf)          # [B,S,D]
    mask32 = np.asarray(mask, np.float32)
    maskb = mask32.astype(bf).ravel()
    # maskt[p, b*NT + t] = mask[b, t*128 + p]
    maskt = np.ascontiguousarray(
        mask32.reshape(B, NT, 128).transpose(2, 0, 1)).astype(bf).ravel()
    pt = (np.asarray(proj, np.float32).T / 8.0).astype(bf)   # [HD, M]
    pt2 = np.concatenate([pt, pt], axis=0).ravel()     # [128*M]
    Ws = {k: np.asarray(w, np.float32).astype(bf)
          for k, w in (("q", Wq), ("k", Wk), ("v", Wv))}
    bs = {k: np.asarray(v, np.float32).astype(bf)
          for k, v in (("q", bq), ("k", bk), ("v", bv))}
    blobs = []
    for c in range(NC):
        cs = slice(c * COLS, (c + 1) * COLS)
        pieces = [
            np.ascontiguousarray(Xb[:, c * SC:(c + 1) * SC, :]).ravel(),
            np.ascontiguousarray(Ws["q"][:, cs]).ravel(),
            np.ascontiguousarray(Ws["k"][:, cs]).ravel(),
            np.ascontiguousarray(Ws["v"][:, cs]).ravel(),
            bs["q"][cs].ravel(), bs["k"][cs].ravel(), bs["v"][cs].ravel(),
            pt2, maskb, maskt,
        ]
        blobs.append({"blob": np.concatenate(pieces)})
    return blobs


class _Runner:
    """Cached jit over the bass custom call (mirrors run_bass_via_pjrt's
    multi-core path, built once). Outputs are donated from the previous
    call's output buffer — the kernel writes every output element."""

    def __init__(self, nc):
        from concourse import bass2jax as b2j
        b2j.install_neuronx_cc_hook()
        self.nc = nc
        fn = nc.m.functions[0]
        pname = nc.partition_id_tensor.name if nc.partition_id_tensor else None
        in_names, out_names, out_avals = [], [], []
        for alloc in fn.allocations:
            if not isinstance(alloc, mybir.MemoryLocationSet):
                continue
            name = alloc.memorylocations[0].name
            if alloc.kind == "ExternalInput":
                if name != pname:
                    in_names.append(name)
            elif alloc.kind == "ExternalOutput":
                assert alloc.tensor_shape is not None
                out_names.append(name)
                out_avals.append(jax.core.ShapedArray(
                    tuple(alloc.tensor_shape), mybir.dt.np(alloc.dtype)))
        self.in_names = list(in_names)
        self.out_names = out_names
        self.out_avals = out_avals
        n_params = len(in_names)
        all_in = tuple(in_names + out_names + ([pname] if pname else []))
        donate = tuple(range(n_params, n_params + len(out_names)))

        def _body(*args):
            operands = list(args)
            if pname is not None:
                operands.append(b2j.partition_id_tensor())
            outs = b2j._bass_exec_p.bind(
                *operands,
                out_avals=tuple(out_avals),
                in_names=all_in,
                out_names=tuple(out_names),
                lowering_input_output_aliases=(),
                sim_require_finite=True,
                sim_require_nnan=True,
                nc=nc,
            )
            return tuple(outs)

        devices = jax.devices()[:NC]
        self.mesh = Mesh(np.asarray(devices), ("core",))
        nin = n_params + len(out_names)
        self.sharding = NamedSharding(self.mesh, PartitionSpec("core"))
        self.jit = jax.jit(
            shard_map(_body, mesh=self.mesh,
                      in_specs=(PartitionSpec("core"),) * nin,
                      out_specs=(PartitionSpec("core"),) * len(out_names),
                      check_rep=False),
            donate_argnums=donate, keep_unused=True)
        self._zeros = None

    def _make_zeros(self):
        outs = []
        for av in self.out_avals:
            shape = (NC * av.shape[0],) + av.shape[1:]
            z = jax.jit(lambda s=shape, d=av.dtype: jax.numpy.zeros(s, d),
                        out_shardings=self.sharding)()
            outs.append(z)
        return outs

    def run(self, per_core_blobs):
        import time as _t
        t0 = _t.perf_counter()
        if self._zeros is None:
            self._zeros = self._make_zeros()
        g_in = np.concatenate(per_core_blobs, axis=0)
        t1 = _t.perf_counter()
        outs = self.jit(g_in, *self._zeros)
        for o in outs:
            o.block_until_ready()
        t2 = _t.perf_counter()
        # recycle output buffers as next call's donated outputs: invalid
        # (donated consumed). Rebuild zeros lazily from outs? outs are the
        # new buffers; they can be donated next call (values overwritten).
        self._zeros = list(outs)
        arr = outs[0]
        shards = sorted(arr.addressable_shards,
                        key=lambda sh: (sh.index[0].start or 0))
        with ThreadPoolExecutor(NC) as ex:
            datas = list(ex.map(lambda sh: np.asarray(sh.data), shards))
        t3 = _t.perf_counter()
        import os
        if os.environ.get("BASSK_DEBUG"):
            print(f"[runner] concat+zeros {t1-t0:.3f}s  jit+block {t2-t1:.3f}s"
                  f"  fetch {t3-t2:.3f}s", flush=True)
        return datas


def _get_runner():
    global _runner
    if _runner is None:
        _runner = _Runner(_build())
    return _runner


def kernel(X, mask, Wq, bq, Wk, bk, Wv, bv, proj):
    r = _get_runner()
    in_maps = _prep_inputs(X, mask, Wq, bq, Wk, bk, Wv, bv, proj)
    datas = r.run([m["blob"] for m in in_maps])
    A = np.concatenate(datas, axis=0).astype(np.float32)   # [D, B*S]
    return A.reshape(D, B, S).transpose(1, 2, 0)
